# revision 6
# baseline (speedup 1.0000x reference)
"""Trainium2 Bass kernel for 16-head MHA (B=2, N=2048, D=1024, H=16).

Sharding: 8 cores = batch(2) x head-group(4). Each core computes 4 heads of
one batch element end-to-end (QKV projections, attention, and its partial
contribution to the output projection). The output projection is a sum over
head features, so each core returns a (N, D) partial product; the host sums
the 4 partials per batch and adds the output bias during unshard.

Per-core dataflow (all matmuls bf16 inputs, f32 PSUM accumulation):
  qT/kT = W @ x.T          (head-feature-major layout, 2 tiles of (128, N))
  v     = x @ Wv.T + bv    stored per key-tile as [v_h | ones] blocks
  scoresT[kt] = k @ q.T    (keys on partitions -> softmax denom comes from PE)
  expT = exp(SCALE*scoresT)  on ScalarE, reading PSUM directly
  [yT; denom] += [v|1].T @ expT  accumulated over key tiles
  yT_norm = yT * 1/denom   (denominator broadcast across partitions by PE)
  partial = yT_norm.T @ WoT
Heads are processed in pairs occupying partitions 0-63 / 64-127 so the two
scoresT matmuls (contraction K=64) row-pack onto disjoint PE row groups.
"""

import os
import sys
from contextlib import ExitStack

import numpy as np

if "/opt/trn_rl_repo" not in sys.path:
    sys.path.insert(0, "/opt/trn_rl_repo")

import ml_dtypes

P = 128
B = 2
NTOK = 2048  # sequence length
D = 1024  # model dim
H_PER_CORE = 4  # heads per core
HD = 64  # head dim
DG = H_PER_CORE * HD  # head-group feature width per core (256)
QB = 512  # query block (matmul free dim)
N_QB = NTOK // QB  # 4
N_KT = NTOK // P  # 16 key tiles
N_DT = D // P  # 8 contraction tiles for projections
SCALE = HD ** -0.5

_BF16 = ml_dtypes.bfloat16


def _emit(tc, t):
    import concourse.bass as bass
    from concourse import mybir

    F32 = mybir.dt.float32
    BF16 = mybir.dt.bfloat16
    Exp = mybir.ActivationFunctionType.Exp
    nc = tc.nc
    CH = 3  # exp-chunk width in 512-col slices (3 PSUM banks)

    with ExitStack() as ctx:
        consts = ctx.enter_context(tc.tile_pool(name="consts", bufs=1))
        misc_psum = ctx.enter_context(
            tc.tile_pool(name="misc_psum", bufs=1, space="PSUM")
        )
        sc_psum = ctx.enter_context(
            tc.tile_pool(name="sc_psum", bufs=2, space="PSUM")
        )
        expT_pool = ctx.enter_context(tc.tile_pool(name="expT", bufs=12))
        rcp_pool = ctx.enter_context(tc.tile_pool(name="rcp", bufs=2))
        ob_pool = ctx.enter_context(tc.tile_pool(name="ob", bufs=3))

        # misc_psum: two alternating single-bank tags shared by projection,
        # PV accumulation, and output-projection psum tiles (sc pool owns the
        # other 6 banks).
        mp_state = [0]

        def mp_tile(shape, name):
            tag = "mp_a" if mp_state[0] % 2 == 0 else "mp_b"
            mp_state[0] += 1
            return misc_psum.tile(shape, F32, tag=tag, name=name)

        # ---- resident SBUF tensors ----
        xT_t = [consts.tile([P, NTOK], BF16, tag=f"xT{i}", name=f"xT{i}") for i in range(N_DT)]
        wqT_t = [consts.tile([P, DG], BF16, tag=f"wqT{i}", name=f"wqT{i}") for i in range(N_DT)]
        wkT_t = [consts.tile([P, DG], BF16, tag=f"wkT{i}", name=f"wkT{i}") for i in range(N_DT)]
        wvT_t = [consts.tile([P, DG], BF16, tag=f"wvT{i}", name=f"wvT{i}") for i in range(N_DT)]
        woT_t = [consts.tile([P, D], BF16, tag=f"woT{i}", name=f"woT{i}") for i in range(DG // P)]
        bq_t = [consts.tile([P, 1], F32, tag=f"bq{i}", name=f"bq{i}") for i in range(DG // P)]
        bk_t = [consts.tile([P, 1], F32, tag=f"bk{i}", name=f"bk{i}") for i in range(DG // P)]
        bvb_t = consts.tile([P, DG], F32, tag="bvb", name="bvb")
        qT_t = [consts.tile([P, NTOK], BF16, tag=f"qT{i}", name=f"qT{i}") for i in range(DG // P)]
        kT_t = [consts.tile([P, NTOK], BF16, tag=f"kT{i}", name=f"kT{i}") for i in range(DG // P)]
        # v per key tile: 4 head blocks of [v_h (64 cols) | ones (64 cols)]
        v_t = [
            consts.tile([P, H_PER_CORE * 2 * HD], BF16, tag=f"v{i}", name=f"v{i}")
            for i in range(N_KT)
        ]
        yT_t = [consts.tile([P, NTOK], BF16, tag=f"yT{i}", name=f"yT{i}") for i in range(DG // P)]

        # ---- ACT exp-table warmup (hides ~2.7us ACT_TABLE_LOAD) ----
        warm = rcp_pool.tile([P, 32], F32, tag="warm", name="warm", bufs=1)
        nc.vector.memset(warm[:], 0.0)
        nc.scalar.activation(warm[:, 0:16], warm[:, 16:32], Exp)

        # ---- input DMAs (weights first, then x column-blocks) ----
        for i in range(N_DT):
            nc.sync.dma_start(wqT_t[i][:], t["wqT"][i * P : (i + 1) * P, :])
            nc.sync.dma_start(wkT_t[i][:], t["wkT"][i * P : (i + 1) * P, :])
            nc.sync.dma_start(wvT_t[i][:], t["wvT"][i * P : (i + 1) * P, :])
        for i in range(DG // P):
            nc.sync.dma_start(woT_t[i][:], t["woT"][i * P : (i + 1) * P, :])
            nc.sync.dma_start(bq_t[i][:], t["bq2"][i])
            nc.sync.dma_start(bk_t[i][:], t["bk2"][i])
        nc.sync.dma_start(bvb_t[:], t["bvb"][:])
        for qb in range(N_QB):
            qsl = slice(qb * QB, (qb + 1) * QB)
            for i in range(N_DT):
                nc.sync.dma_start(xT_t[i][:, qsl], t["xT"][i * P : (i + 1) * P, qsl])

        # ---- projections: qk for head pair 0, then v, then qk pair 1 ----
        def emit_qk(pt):
            for wt, bt, dst in ((wqT_t, bq_t, qT_t), (wkT_t, bk_t, kT_t)):
                for qb in range(N_QB):
                    pp = mp_tile([P, QB], "pp_qk")
                    for dt in range(N_DT):
                        nc.tensor.matmul(
                            pp[:],
                            lhsT=wt[dt][:, pt * P : (pt + 1) * P],
                            rhs=xT_t[dt][:, qb * QB : (qb + 1) * QB],
                            start=(dt == 0),
                            stop=(dt == N_DT - 1),
                        )
                    nc.vector.tensor_scalar_add(
                        dst[pt][:, qb * QB : (qb + 1) * QB], pp[:], bt[pt][:]
                    )

        def emit_v():
            for kt in range(N_KT):
                pp = mp_tile([P, DG], "pp_v")
                for dt in range(N_DT):
                    nc.tensor.matmul(
                        pp[:],
                        lhsT=xT_t[dt][:, kt * P : (kt + 1) * P],
                        rhs=wvT_t[dt][:],
                        start=(dt == 0),
                        stop=(dt == N_DT - 1),
                    )
                vk = v_t[kt].rearrange("p (h w) -> p h w", h=H_PER_CORE)
                nc.vector.tensor_add(
                    vk[:, :, 0:HD],
                    pp[:].rearrange("p (h w) -> p h w", h=H_PER_CORE),
                    bvb_t[:].rearrange("p (h w) -> p h w", h=H_PER_CORE),
                )
                nc.vector.memset(vk[:, :, HD : 2 * HD], 1.0)

        emit_qk(0)
        emit_v()
        emit_qk(1)

        # ---- attention for one (head pair, query block) ----
        def attention(pair, qb):
            kt_p = kT_t[pair]
            qt_p = qT_t[pair]
            qsl = slice(qb * QB, (qb + 1) * QB)
            pv = [mp_tile([P, QB], "pv_a"), mp_tile([P, QB], "pv_b")]
            slices = [(kt, h) for kt in range(N_KT) for h in (0, 1)]
            sc = ex = None
            for idx, (kt, hsel) in enumerate(slices):
                cj = idx % CH
                if cj == 0:
                    sc = sc_psum.tile([P, CH * QB], F32, tag="sc", name="sc")
                    ex = expT_pool.tile([P, CH * QB], BF16, tag="ex", name="ex")
                ksl = slice(kt * P, (kt + 1) * P)
                po = hsel * HD
                # scoresT: adjacent head-pair matmuls pack onto disjoint
                # PE row groups (contraction K=64 each)
                nc.tensor.matmul(
                    sc[:, cj * QB : (cj + 1) * QB],
                    lhsT=kt_p[po : po + HD, ksl],
                    rhs=qt_p[po : po + HD, qsl],
                    start=True,
                    stop=True,
                )
                if cj == CH - 1 or idx == len(slices) - 1:
                    w = (cj + 1) * QB
                    nc.scalar.activation(ex[:, 0:w], sc[:, 0:w], Exp, scale=SCALE)
                    for j in range(cj + 1):
                        kt2, h2 = slices[idx - cj + j]
                        head = 2 * pair + h2
                        nc.tensor.matmul(
                            pv[h2][:],
                            lhsT=v_t[kt2][:, head * 2 * HD : (head + 1) * 2 * HD],
                            rhs=ex[:, j * QB : (j + 1) * QB],
                            start=(kt2 == 0),
                            stop=(kt2 == N_KT - 1),
                        )
            # normalize: yT rows 0-63, PE-broadcast denominator rows 64-127
            for h2 in (0, 1):
                rc = rcp_pool.tile([P, QB], F32, tag="rc", name="rc")
                nc.vector.reciprocal(rc[HD:P, :], pv[h2][HD:P, :])
                nc.vector.tensor_mul(
                    yT_t[pair][h2 * HD : (h2 + 1) * HD, qsl],
                    pv[h2][0:HD, :],
                    rc[HD:P, :],
                )

        # ---- output projection for one query block's token rows ----
        def oproj(qb):
            for mt in range(4 * qb, 4 * qb + 4):
                msl = slice(mt * P, (mt + 1) * P)
                ob = ob_pool.tile([P, D], F32, tag="ob", name="ob")
                for nb in range(D // QB):
                    op = mp_tile([P, QB], "op")
                    for ktile in range(DG // P):
                        nc.tensor.matmul(
                            op[:],
                            lhsT=yT_t[ktile][:, msl],
                            rhs=woT_t[ktile][:, nb * QB : (nb + 1) * QB],
                            start=(ktile == 0),
                            stop=(ktile == DG // P - 1),
                        )
                    nc.vector.tensor_copy(ob[:, nb * QB : (nb + 1) * QB], op[:])
                nc.sync.dma_start(t["partial"][msl, :], ob[:])

        for qb in range(N_QB):
            attention(0, qb)
        for qb in range(N_QB):
            attention(1, qb)
            oproj(qb)


def _build():
    import concourse.bacc as bacc
    import concourse.tile as tile
    from concourse import mybir

    F32 = mybir.dt.float32
    BF16 = mybir.dt.bfloat16

    nc = bacc.Bacc(
        "TRN2", target_bir_lowering=False, debug=False, num_devices=8
    )
    t = {
        "xT": nc.dram_tensor("xT", (D, NTOK), BF16, kind="ExternalInput").ap(),
        "wqT": nc.dram_tensor("wqT", (D, DG), BF16, kind="ExternalInput").ap(),
        "wkT": nc.dram_tensor("wkT", (D, DG), BF16, kind="ExternalInput").ap(),
        "wvT": nc.dram_tensor("wvT", (D, DG), BF16, kind="ExternalInput").ap(),
        "woT": nc.dram_tensor("woT", (DG, D), BF16, kind="ExternalInput").ap(),
        "bq2": nc.dram_tensor(
            "bq2", (DG // P, P, 1), F32, kind="ExternalInput"
        ).ap(),
        "bk2": nc.dram_tensor(
            "bk2", (DG // P, P, 1), F32, kind="ExternalInput"
        ).ap(),
        "bvb": nc.dram_tensor("bvb", (P, DG), F32, kind="ExternalInput").ap(),
        "partial": nc.dram_tensor(
            "partial", (NTOK, D), F32, kind="ExternalOutput"
        ).ap(),
    }
    with tile.TileContext(nc) as tc:
        _emit(tc, t)
    nc.compile()
    return nc


_CACHE = {}


def _get_nc():
    if "nc" not in _CACHE:
        _CACHE["nc"] = _build()
    return _CACHE["nc"]


def make_in_maps(x, Wq, bq, Wk, bk, Wv, bv, Wo):
    """Per-core host-side sharding: core c -> batch c//4, head group c%4."""
    in_maps = []
    for c in range(8):
        b, g = divmod(c, 4)
        sl = slice(DG * g, DG * (g + 1))
        in_maps.append(
            {
                "xT": np.ascontiguousarray(x[b].T).astype(_BF16),
                "wqT": np.ascontiguousarray(Wq[sl].T).astype(_BF16),
                "wkT": np.ascontiguousarray(Wk[sl].T).astype(_BF16),
                "wvT": np.ascontiguousarray(Wv[sl].T).astype(_BF16),
                "woT": np.ascontiguousarray(Wo[:, sl].T).astype(_BF16),
                "bq2": np.ascontiguousarray(
                    bq[sl].reshape(DG // P, P, 1)
                ).astype(np.float32),
                "bk2": np.ascontiguousarray(
                    bk[sl].reshape(DG // P, P, 1)
                ).astype(np.float32),
                "bvb": np.ascontiguousarray(
                    np.broadcast_to(bv[sl][None, :], (P, DG))
                ).astype(np.float32),
            }
        )
    return in_maps


def kernel(x, Wq, bq, Wk, bk, Wv, bv, Wo, bo, _spmd_kwargs=None):
    from concourse.bass_utils import run_bass_kernel_spmd

    x, Wq, bq, Wk, bk, Wv, bv, Wo, bo = (
        np.asarray(a, np.float32) for a in (x, Wq, bq, Wk, bk, Wv, bv, Wo, bo)
    )
    nc = _get_nc()
    in_maps = make_in_maps(x, Wq, bq, Wk, bk, Wv, bv, Wo)
    res = run_bass_kernel_spmd(
        nc, in_maps, list(range(8)), **(_spmd_kwargs or {})
    )
    _CACHE["last_results"] = res
    out = np.empty((B, NTOK, D), np.float32)
    for b in range(B):
        acc = res.results[4 * b]["partial"].astype(np.float32).copy()
        for g in range(1, 4):
            acc += res.results[4 * b + g]["partial"]
        out[b] = acc + bo[None, :]
    return out


# revision 11
# speedup vs baseline: 1.1597x; 1.1597x over previous
"""Trainium2 Bass kernel for 16-head MHA (B=2, N=2048, D=1024, H=16).

Sharding: 8 cores = batch(2) x head-group(4). Each core computes 4 heads of
one batch element end-to-end (QKV projections, attention, and its partial
contribution to the output projection). The output projection is a sum over
head features, so each core returns a (N, D) partial product; the host sums
the 4 partials per batch and adds the output bias during unshard.

Per-core dataflow (all matmuls bf16 inputs, f32 PSUM accumulation):
  qT/kT = W @ x.T          (head-feature-major layout, 2 tiles of (128, N))
  v     = x @ Wv.T + bv    stored per key-tile as [v_h | ones] blocks
  scoresT[kt] = k @ q.T    (keys on partitions -> softmax denom comes from PE)
  expT = exp(SCALE*scoresT)  on ScalarE, reading PSUM directly
  [yT; denom] += [v|1].T @ expT  accumulated over key tiles
  yT_norm = yT * 1/denom   (denominator broadcast across partitions by PE)
  partial = yT_norm.T @ WoT
Heads are processed in pairs occupying partitions 0-63 / 64-127 so the two
scoresT matmuls (contraction K=64) row-pack onto disjoint PE row groups.
"""

import os
import sys
from contextlib import ExitStack

import numpy as np

if "/opt/trn_rl_repo" not in sys.path:
    sys.path.insert(0, "/opt/trn_rl_repo")

import ml_dtypes

P = 128
B = 2
NTOK = 2048  # sequence length
D = 1024  # model dim
H_PER_CORE = 4  # heads per core
HD = 64  # head dim
DG = H_PER_CORE * HD  # head-group feature width per core (256)
QB = 512  # query block (matmul free dim)
N_QB = NTOK // QB  # 4
N_KT = NTOK // P  # 16 key tiles
N_DT = D // P  # 8 contraction tiles for projections
SCALE = HD ** -0.5

_BF16 = ml_dtypes.bfloat16


NORM_ON_GPSIMD = False


def _emit(tc, t):
    import concourse.bass as bass
    from concourse import mybir

    F32 = mybir.dt.float32
    BF16 = mybir.dt.bfloat16
    Exp = mybir.ActivationFunctionType.Exp
    nc = tc.nc

    with ExitStack() as ctx:
        consts = ctx.enter_context(tc.tile_pool(name="consts", bufs=1))
        # PSUM budget (8 banks): sc 2x2 + pv 2x1 + pp 2x1 = 8
        pp_psum = ctx.enter_context(
            tc.tile_pool(name="pp_psum", bufs=2, space="PSUM")
        )
        sc_psum = ctx.enter_context(
            tc.tile_pool(name="sc_psum", bufs=2, space="PSUM")
        )
        pv_psum = ctx.enter_context(
            tc.tile_pool(name="pv_psum", bufs=1, space="PSUM")
        )
        expT_pool = ctx.enter_context(tc.tile_pool(name="expT", bufs=6))
        nrm_pool = ctx.enter_context(tc.tile_pool(name="nrm", bufs=4))
        ob_pool = ctx.enter_context(tc.tile_pool(name="ob", bufs=3))

        # ---- resident SBUF tensors (single tiles, one DMA each) ----
        xT_t = consts.tile([P, N_DT * NTOK], BF16, tag="xT", name="xT")
        wq_t = consts.tile([P, N_DT * DG], BF16, tag="wq", name="wq")
        wk_t = consts.tile([P, N_DT * DG], BF16, tag="wk", name="wk")
        wv_t = consts.tile([P, N_DT * DG], BF16, tag="wv", name="wv")
        wo_t = consts.tile([P, (DG // P) * D], BF16, tag="wo", name="wo")
        bq_t = consts.tile([P, DG // P], F32, tag="bq", name="bq")
        bk_t = consts.tile([P, DG // P], F32, tag="bk", name="bk")
        bvb_t = consts.tile([P, DG], F32, tag="bvb", name="bvb")
        qT_t = [consts.tile([P, NTOK], BF16, tag=f"qT{i}", name=f"qT{i}") for i in range(DG // P)]
        kT_t = [consts.tile([P, NTOK], BF16, tag=f"kT{i}", name=f"kT{i}") for i in range(DG // P)]
        v_t = [
            consts.tile([P, H_PER_CORE * 2 * HD], BF16, tag=f"v{i}", name=f"v{i}")
            for i in range(N_KT)
        ]
        yT_t = [consts.tile([P, NTOK], BF16, tag=f"yT{i}", name=f"yT{i}") for i in range(DG // P)]

        # ---- ACT exp-table warmup (hides ~2.7us ACT_TABLE_LOAD) ----
        warm = nrm_pool.tile([P, 32], F32, tag="warm", name="warm", bufs=1)
        nc.vector.memset(warm[:], 0.0)
        nc.scalar.activation(warm[:, 0:16], warm[:, 16:32], Exp)

        # ---- input DMAs: one big descriptor per tensor ----
        nc.sync.dma_start(
            wq_t.rearrange("p (a d) -> p a d", a=N_DT),
            t["wqT"].rearrange("(a p) d -> p a d", p=P),
        )
        nc.sync.dma_start(
            wk_t.rearrange("p (a d) -> p a d", a=N_DT),
            t["wkT"].rearrange("(a p) d -> p a d", p=P),
        )
        nc.sync.dma_start(
            wv_t.rearrange("p (a d) -> p a d", a=N_DT),
            t["wvT"].rearrange("(a p) d -> p a d", p=P),
        )
        nc.sync.dma_start(
            wo_t.rearrange("p (a d) -> p a d", a=DG // P),
            t["woT"].rearrange("(a p) d -> p a d", p=P),
        )
        nc.sync.dma_start(bq_t[:], t["bq2"][:])
        nc.sync.dma_start(bk_t[:], t["bk2"][:])
        nc.sync.dma_start(bvb_t[:], t["bvb"][:])
        xT_v = xT_t.rearrange("p (a n) -> p a n", a=N_DT)
        xT_d = t["xT"].rearrange("(a p) n -> p a n", p=P)
        for qb in range(N_QB):
            qsl = slice(qb * QB, (qb + 1) * QB)
            nc.sync.dma_start(xT_v[:, :, qsl], xT_d[:, :, qsl])

        # slice helpers into the packed single tiles
        def w_sl(wt, dt, pt):
            o = dt * DG + pt * P
            return wt[:, o : o + P]

        def x_sl(dt, lo, hi):
            return xT_t[:, dt * NTOK + lo : dt * NTOK + hi]

        # ---- projection group emitters (each: 8 matmuls + eviction) ----
        def qk_group(wt, bt, dst, pt, qb):
            pp = pp_psum.tile([P, QB], F32, tag="pp", name="pp")
            for dt in range(N_DT):
                nc.tensor.matmul(
                    pp[:],
                    lhsT=w_sl(wt, dt, pt),
                    rhs=x_sl(dt, qb * QB, (qb + 1) * QB),
                    start=(dt == 0),
                    stop=(dt == N_DT - 1),
                )
            nc.vector.tensor_scalar_add(
                dst[pt][:, qb * QB : (qb + 1) * QB], pp[:], bt[:, pt : pt + 1]
            )

        def v_group(kt):
            pp = pp_psum.tile([P, QB], F32, tag="pp", name="pp_v")
            for dt in range(N_DT):
                nc.tensor.matmul(
                    pp[:, 0:DG],
                    lhsT=x_sl(dt, kt * P, (kt + 1) * P),
                    rhs=wv_t[:, dt * DG : (dt + 1) * DG],
                    start=(dt == 0),
                    stop=(dt == N_DT - 1),
                )
            vk = v_t[kt].rearrange("p (h w) -> p h w", h=H_PER_CORE)
            nc.vector.tensor_add(
                vk[:, :, 0:HD],
                pp[:, 0:DG].rearrange("p (h w) -> p h w", h=H_PER_CORE),
                bvb_t[:].rearrange("p (h w) -> p h w", h=H_PER_CORE),
            )
            nc.vector.memset(vk[:, :, HD : 2 * HD], 1.0)

        # ---- attention for one (head pair, query block) ----
        # fillers: chunk index -> list of projection-group thunks, woven into
        # the PE stream so proj work hides under the exp-bound steady state
        def attention(pair, qb, fillers={}):
            kt_p = kT_t[pair]
            qt_p = qT_t[pair]
            qsl = slice(qb * QB, (qb + 1) * QB)
            pv = [
                pv_psum.tile([P, QB], F32, tag="pv_a", name="pv_a"),
                pv_psum.tile([P, QB], F32, tag="pv_b", name="pv_b"),
            ]
            for kt in range(N_KT):
                for f in fillers.get(kt, ()):
                    f()
                ksl = slice(kt * P, (kt + 1) * P)
                sc = sc_psum.tile([P, 2 * QB], F32, tag="sc", name="sc")
                # scoresT: the head-pair matmuls (K=64 each) row-pack onto
                # disjoint PE row groups and run concurrently
                nc.tensor.matmul(
                    sc[:, 0:QB],
                    lhsT=kt_p[0:HD, ksl],
                    rhs=qt_p[0:HD, qsl],
                    start=True,
                    stop=True,
                )
                nc.tensor.matmul(
                    sc[:, QB : 2 * QB],
                    lhsT=kt_p[HD:P, ksl],
                    rhs=qt_p[HD:P, qsl],
                    start=True,
                    stop=True,
                )
                ex = expT_pool.tile([P, 2 * QB], BF16, tag="ex", name="ex")
                nc.scalar.activation(ex[:], sc[:], Exp, scale=SCALE)
                for h2 in (0, 1):
                    head = 2 * pair + h2
                    nc.tensor.matmul(
                        pv[h2][:],
                        lhsT=v_t[kt][:, head * 2 * HD : (head + 1) * 2 * HD],
                        rhs=ex[:, h2 * QB : (h2 + 1) * QB],
                        start=(kt == 0),
                        stop=(kt == N_KT - 1),
                    )
            # normalize: copy PSUM out fast (frees pv slots), then divide
            # y rows 0-63 by the PE-broadcast denominator rows 64-127
            for h2 in (0, 1):
                cpy = nrm_pool.tile([HD, QB], F32, tag="cpy", name="cpy")
                cpd = nrm_pool.tile([HD, QB], F32, tag="cpd", name="cpd")
                nc.vector.tensor_copy(cpy[:], pv[h2][0:HD, :])
                nc.vector.tensor_copy(cpd[:], pv[h2][HD:P, :])
                dst = yT_t[pair][h2 * HD : (h2 + 1) * HD, qsl]
                if NORM_ON_GPSIMD:
                    nc.gpsimd.tensor_tensor(
                        dst, cpy[:], cpd[:], op=mybir.AluOpType.divide
                    )
                else:
                    rc = nrm_pool.tile([HD, QB], F32, tag="rc", name="rc")
                    nc.vector.reciprocal(rc[:], cpd[:])
                    nc.vector.tensor_mul(dst, cpy[:], rc[:])

        # ---- output projection for one query block's token rows ----
        def oproj(qb):
            for mt in range(4 * qb, 4 * qb + 4):
                msl = slice(mt * P, (mt + 1) * P)
                ob = ob_pool.tile([P, D], F32, tag="ob", name="ob")
                for nb in range(D // QB):
                    op = pp_psum.tile([P, QB], F32, tag="pp", name="op")
                    for ktile in range(DG // P):
                        nc.tensor.matmul(
                            op[:],
                            lhsT=yT_t[ktile][:, msl],
                            rhs=wo_t[:, ktile * D + nb * QB : ktile * D + (nb + 1) * QB],
                            start=(ktile == 0),
                            stop=(ktile == DG // P - 1),
                        )
                    nc.vector.tensor_copy(ob[:, nb * QB : (nb + 1) * QB], op[:])
                nc.sync.dma_start(t["partial"][msl, :], ob[:])

        # ---- schedule ----
        # head: k columns (all query blocks) + q columns for qb0 of pair 0
        for qb in range(N_QB):
            qk_group(wk_t, bk_t, kT_t, 0, qb)
        qk_group(wq_t, bq_t, qT_t, 0, 0)

        G = lambda *a: (lambda: qk_group(*a))
        V = lambda kt: (lambda: v_group(kt))
        attention(0, 0, {
            0: [V(0), V(1)], 1: [V(2), V(3)], 2: [V(4), V(5)], 3: [V(6), V(7)],
            4: [V(8), V(9)], 5: [V(10), V(11)], 6: [V(12), V(13)], 7: [V(14), V(15)],
            10: [G(wq_t, bq_t, qT_t, 0, 1)],
            12: [G(wq_t, bq_t, qT_t, 0, 2)],
            14: [G(wq_t, bq_t, qT_t, 0, 3)],
        })
        attention(0, 1, {
            2: [G(wk_t, bk_t, kT_t, 1, 0)],
            6: [G(wk_t, bk_t, kT_t, 1, 1)],
            10: [G(wk_t, bk_t, kT_t, 1, 2)],
            14: [G(wk_t, bk_t, kT_t, 1, 3)],
        })
        attention(0, 2, {
            2: [G(wq_t, bq_t, qT_t, 1, 0)],
            6: [G(wq_t, bq_t, qT_t, 1, 1)],
            10: [G(wq_t, bq_t, qT_t, 1, 2)],
            14: [G(wq_t, bq_t, qT_t, 1, 3)],
        })
        attention(0, 3)
        for qb in range(N_QB):
            attention(1, qb)
            oproj(qb)


def _build():
    import concourse.bacc as bacc
    import concourse.tile as tile
    from concourse import mybir

    F32 = mybir.dt.float32
    BF16 = mybir.dt.bfloat16

    nc = bacc.Bacc(
        "TRN2", target_bir_lowering=False, debug=False, num_devices=8
    )
    t = {
        "xT": nc.dram_tensor("xT", (D, NTOK), BF16, kind="ExternalInput").ap(),
        "wqT": nc.dram_tensor("wqT", (D, DG), BF16, kind="ExternalInput").ap(),
        "wkT": nc.dram_tensor("wkT", (D, DG), BF16, kind="ExternalInput").ap(),
        "wvT": nc.dram_tensor("wvT", (D, DG), BF16, kind="ExternalInput").ap(),
        "woT": nc.dram_tensor("woT", (DG, D), BF16, kind="ExternalInput").ap(),
        "bq2": nc.dram_tensor(
            "bq2", (P, DG // P), F32, kind="ExternalInput"
        ).ap(),
        "bk2": nc.dram_tensor(
            "bk2", (P, DG // P), F32, kind="ExternalInput"
        ).ap(),
        "bvb": nc.dram_tensor("bvb", (P, DG), F32, kind="ExternalInput").ap(),
        "partial": nc.dram_tensor(
            "partial", (NTOK, D), F32, kind="ExternalOutput"
        ).ap(),
    }
    with tile.TileContext(nc) as tc:
        _emit(tc, t)
    nc.compile()
    return nc


_CACHE = {}


def _get_nc():
    if "nc" not in _CACHE:
        _CACHE["nc"] = _build()
    return _CACHE["nc"]


def make_in_maps(x, Wq, bq, Wk, bk, Wv, bv, Wo):
    """Per-core host-side sharding: core c -> batch c//4, head group c%4."""
    in_maps = []
    for c in range(8):
        b, g = divmod(c, 4)
        sl = slice(DG * g, DG * (g + 1))
        in_maps.append(
            {
                "xT": np.ascontiguousarray(x[b].T).astype(_BF16),
                "wqT": np.ascontiguousarray(Wq[sl].T).astype(_BF16),
                "wkT": np.ascontiguousarray(Wk[sl].T).astype(_BF16),
                "wvT": np.ascontiguousarray(Wv[sl].T).astype(_BF16),
                "woT": np.ascontiguousarray(Wo[:, sl].T).astype(_BF16),
                "bq2": np.ascontiguousarray(
                    bq[sl].reshape(DG // P, P).T
                ).astype(np.float32),
                "bk2": np.ascontiguousarray(
                    bk[sl].reshape(DG // P, P).T
                ).astype(np.float32),
                "bvb": np.ascontiguousarray(
                    np.broadcast_to(bv[sl][None, :], (P, DG))
                ).astype(np.float32),
            }
        )
    return in_maps


def kernel(x, Wq, bq, Wk, bk, Wv, bv, Wo, bo, _spmd_kwargs=None):
    from concourse.bass_utils import run_bass_kernel_spmd

    x, Wq, bq, Wk, bk, Wv, bv, Wo, bo = (
        np.asarray(a, np.float32) for a in (x, Wq, bq, Wk, bk, Wv, bv, Wo, bo)
    )
    nc = _get_nc()
    in_maps = make_in_maps(x, Wq, bq, Wk, bk, Wv, bv, Wo)
    res = run_bass_kernel_spmd(
        nc, in_maps, list(range(8)), **(_spmd_kwargs or {})
    )
    _CACHE["last_results"] = res
    out = np.empty((B, NTOK, D), np.float32)
    for b in range(B):
        acc = res.results[4 * b]["partial"].astype(np.float32).copy()
        for g in range(1, 4):
            acc += res.results[4 * b + g]["partial"]
        out[b] = acc + bo[None, :]
    return out


# revision 12
# speedup vs baseline: 1.2157x; 1.0482x over previous
"""Trainium2 Bass kernel for 16-head MHA (B=2, N=2048, D=1024, H=16).

Sharding: 8 cores = batch(2) x head-group(4). Each core computes 4 heads of
one batch element end-to-end (QKV projections, attention, and its partial
contribution to the output projection). The output projection is a sum over
head features, so each core returns a (N, D) partial product; the host sums
the 4 partials per batch and adds the output bias during unshard.

Per-core dataflow (all matmuls bf16 inputs, f32 PSUM accumulation):
  qT/kT = W @ x.T          (head-feature-major layout, 2 tiles of (128, N))
  v     = x @ Wv.T + bv    stored per key-tile as [v_h | ones] blocks
  scoresT[kt] = k @ q.T    (keys on partitions -> softmax denom comes from PE)
  expT = exp(SCALE*scoresT)  on ScalarE, reading PSUM directly
  [yT; denom] += [v|1].T @ expT  accumulated over key tiles
  yT_norm = yT * 1/denom   (denominator broadcast across partitions by PE)
  partial = yT_norm.T @ WoT
Heads are processed in pairs occupying partitions 0-63 / 64-127 so the two
scoresT matmuls (contraction K=64) row-pack onto disjoint PE row groups.
"""

import os
import sys
from contextlib import ExitStack

import numpy as np

if "/opt/trn_rl_repo" not in sys.path:
    sys.path.insert(0, "/opt/trn_rl_repo")

import ml_dtypes

P = 128
B = 2
NTOK = 2048  # sequence length
D = 1024  # model dim
H_PER_CORE = 4  # heads per core
HD = 64  # head dim
DG = H_PER_CORE * HD  # head-group feature width per core (256)
QB = 512  # query block (matmul free dim)
N_QB = NTOK // QB  # 4
N_KT = NTOK // P  # 16 key tiles
N_DT = D // P  # 8 contraction tiles for projections
SCALE = HD ** -0.5

_BF16 = ml_dtypes.bfloat16


NORM_ON_GPSIMD = False


def _emit(tc, t):
    import concourse.bass as bass
    from concourse import mybir

    F32 = mybir.dt.float32
    BF16 = mybir.dt.bfloat16
    Exp = mybir.ActivationFunctionType.Exp
    nc = tc.nc

    with ExitStack() as ctx:
        consts = ctx.enter_context(tc.tile_pool(name="consts", bufs=1))
        # PSUM budget (8 banks): sc 2x2 + pv 2x1 + pp 2x1 = 8
        pp_psum = ctx.enter_context(
            tc.tile_pool(name="pp_psum", bufs=2, space="PSUM")
        )
        sc_psum = ctx.enter_context(
            tc.tile_pool(name="sc_psum", bufs=2, space="PSUM")
        )
        pv_psum = ctx.enter_context(
            tc.tile_pool(name="pv_psum", bufs=1, space="PSUM")
        )
        expT_pool = ctx.enter_context(tc.tile_pool(name="expT", bufs=6))
        nrm_pool = ctx.enter_context(tc.tile_pool(name="nrm", bufs=4))
        ob_pool = ctx.enter_context(tc.tile_pool(name="ob", bufs=3))

        # ---- resident SBUF tensors (single tiles, one DMA each) ----
        xT_t = consts.tile([P, N_DT * NTOK], BF16, tag="xT", name="xT")
        wq_t = consts.tile([P, N_DT * DG], BF16, tag="wq", name="wq")
        wk_t = consts.tile([P, N_DT * DG], BF16, tag="wk", name="wk")
        wv_t = consts.tile([P, N_DT * DG], BF16, tag="wv", name="wv")
        wo_t = consts.tile([P, (DG // P) * D], BF16, tag="wo", name="wo")
        bq_t = consts.tile([P, DG // P], F32, tag="bq", name="bq")
        bk_t = consts.tile([P, DG // P], F32, tag="bk", name="bk")
        bvb_t = consts.tile([P, DG], F32, tag="bvb", name="bvb")
        qT_t = [consts.tile([P, NTOK], BF16, tag=f"qT{i}", name=f"qT{i}") for i in range(DG // P)]
        kT_t = [consts.tile([P, NTOK], BF16, tag=f"kT{i}", name=f"kT{i}") for i in range(DG // P)]
        v_t = [
            consts.tile([P, H_PER_CORE * 2 * HD], BF16, tag=f"v{i}", name=f"v{i}")
            for i in range(N_KT)
        ]
        yT_t = [consts.tile([P, NTOK], BF16, tag=f"yT{i}", name=f"yT{i}") for i in range(DG // P)]

        # ---- ACT exp-table warmup (hides ~2.7us ACT_TABLE_LOAD) ----
        warm = nrm_pool.tile([P, 32], F32, tag="warm", name="warm", bufs=1)
        nc.vector.memset(warm[:], 0.0)
        nc.scalar.activation(warm[:, 0:16], warm[:, 16:32], Exp)

        # ---- input DMAs: one big descriptor per tensor ----
        nc.sync.dma_start(
            wq_t.rearrange("p (a d) -> p a d", a=N_DT),
            t["wqT"].rearrange("(a p) d -> p a d", p=P),
        )
        nc.sync.dma_start(
            wk_t.rearrange("p (a d) -> p a d", a=N_DT),
            t["wkT"].rearrange("(a p) d -> p a d", p=P),
        )
        nc.sync.dma_start(
            wv_t.rearrange("p (a d) -> p a d", a=N_DT),
            t["wvT"].rearrange("(a p) d -> p a d", p=P),
        )
        nc.sync.dma_start(
            wo_t.rearrange("p (a d) -> p a d", a=DG // P),
            t["woT"].rearrange("(a p) d -> p a d", p=P),
        )
        nc.sync.dma_start(bq_t[:], t["bq2"][:])
        nc.sync.dma_start(bk_t[:], t["bk2"][:])
        nc.sync.dma_start(bvb_t[:], t["bvb"][:])
        xT_v = xT_t.rearrange("p (a n) -> p a n", a=N_DT)
        xT_d = t["xT"].rearrange("(a p) n -> p a n", p=P)
        for qb in range(N_QB):
            qsl = slice(qb * QB, (qb + 1) * QB)
            nc.sync.dma_start(xT_v[:, :, qsl], xT_d[:, :, qsl])

        # slice helpers into the packed single tiles
        def w_sl(wt, dt, pt):
            o = dt * DG + pt * P
            return wt[:, o : o + P]

        def x_sl(dt, lo, hi):
            return xT_t[:, dt * NTOK + lo : dt * NTOK + hi]

        # ---- projection group emitters (each: 8 matmuls + eviction) ----
        def qk_group(wt, bt, dst, pt, qb):
            pp = pp_psum.tile([P, QB], F32, tag="pp", name="pp")
            for dt in range(N_DT):
                nc.tensor.matmul(
                    pp[:],
                    lhsT=w_sl(wt, dt, pt),
                    rhs=x_sl(dt, qb * QB, (qb + 1) * QB),
                    start=(dt == 0),
                    stop=(dt == N_DT - 1),
                )
            nc.vector.tensor_scalar_add(
                dst[pt][:, qb * QB : (qb + 1) * QB], pp[:], bt[:, pt : pt + 1]
            )

        def v_group(kt):
            pp = pp_psum.tile([P, QB], F32, tag="pp", name="pp_v")
            for dt in range(N_DT):
                nc.tensor.matmul(
                    pp[:, 0:DG],
                    lhsT=x_sl(dt, kt * P, (kt + 1) * P),
                    rhs=wv_t[:, dt * DG : (dt + 1) * DG],
                    start=(dt == 0),
                    stop=(dt == N_DT - 1),
                )
            vk = v_t[kt].rearrange("p (h w) -> p h w", h=H_PER_CORE)
            nc.vector.tensor_add(
                vk[:, :, 0:HD],
                pp[:, 0:DG].rearrange("p (h w) -> p h w", h=H_PER_CORE),
                bvb_t[:].rearrange("p (h w) -> p h w", h=H_PER_CORE),
            )
            nc.vector.memset(vk[:, :, HD : 2 * HD], 1.0)

        # ---- attention for one (head pair, query block) ----
        # LAG-1 software pipeline: SC(kt+1) and filler work are emitted
        # between exp(kt) and PV(kt) so the PE never idles waiting on the
        # ScalarE exp. fillers: chunk -> projection/oproj thunks.
        def attention(pair, qb, fillers={}, pre=()):
            kt_p = kT_t[pair]
            qt_p = qT_t[pair]
            qsl = slice(qb * QB, (qb + 1) * QB)
            pv = [
                pv_psum.tile([P, QB], F32, tag="pv_a", name="pv_a"),
                pv_psum.tile([P, QB], F32, tag="pv_b", name="pv_b"),
            ]

            ex_t = {}

            def sc_exp(kt):
                ksl = slice(kt * P, (kt + 1) * P)
                sc = sc_psum.tile([P, 2 * QB], F32, tag="sc", name="sc")
                # the head-pair matmuls (K=64 each) row-pack onto disjoint
                # PE row groups and run concurrently
                nc.tensor.matmul(
                    sc[:, 0:QB],
                    lhsT=kt_p[0:HD, ksl],
                    rhs=qt_p[0:HD, qsl],
                    start=True,
                    stop=True,
                )
                nc.tensor.matmul(
                    sc[:, QB : 2 * QB],
                    lhsT=kt_p[HD:P, ksl],
                    rhs=qt_p[HD:P, qsl],
                    start=True,
                    stop=True,
                )
                ex = expT_pool.tile([P, 2 * QB], BF16, tag="ex", name="ex")
                nc.scalar.activation(ex[:], sc[:], Exp, scale=SCALE)
                ex_t[kt] = ex

            sc_exp(0)
            for f in pre:
                f()
            for kt in range(N_KT):
                if kt + 1 < N_KT:
                    sc_exp(kt + 1)
                for f in fillers.get(kt, ()):
                    f()
                ex = ex_t.pop(kt)
                for h2 in (0, 1):
                    head = 2 * pair + h2
                    nc.tensor.matmul(
                        pv[h2][:],
                        lhsT=v_t[kt][:, head * 2 * HD : (head + 1) * 2 * HD],
                        rhs=ex[:, h2 * QB : (h2 + 1) * QB],
                        start=(kt == 0),
                        stop=(kt == N_KT - 1),
                    )
            # normalize: copy both halves out of PSUM fast (frees pv slots),
            # then 1/denominator * y off the critical path
            for h2 in (0, 1):
                cpy = nrm_pool.tile([HD, QB], F32, tag="cpy", name="cpy")
                cpd = nrm_pool.tile([HD, QB], F32, tag="cpd", name="cpd")
                nc.vector.tensor_copy(cpy[:], pv[h2][0:HD, :])
                nc.vector.tensor_copy(cpd[:], pv[h2][HD:P, :])
                dst = yT_t[pair][h2 * HD : (h2 + 1) * HD, qsl]
                rc = nrm_pool.tile([HD, QB], F32, tag="rc", name="rc")
                nc.vector.reciprocal(rc[:], cpd[:])
                nc.vector.tensor_mul(dst, cpy[:], rc[:])

        # ---- output projection for one token tile (128 rows) ----
        def oproj_mt(mt):
            msl = slice(mt * P, (mt + 1) * P)
            ob = ob_pool.tile([P, D], F32, tag="ob", name="ob")
            for nb in range(D // QB):
                op = pp_psum.tile([P, QB], F32, tag="pp", name="op")
                for ktile in range(DG // P):
                    nc.tensor.matmul(
                        op[:],
                        lhsT=yT_t[ktile][:, msl],
                        rhs=wo_t[:, ktile * D + nb * QB : ktile * D + (nb + 1) * QB],
                        start=(ktile == 0),
                        stop=(ktile == DG // P - 1),
                    )
                nc.vector.tensor_copy(ob[:, nb * QB : (nb + 1) * QB], op[:])
            nc.sync.dma_start(t["partial"][msl, :], ob[:])

        # ---- schedule ----
        G = lambda *a: (lambda: qk_group(*a))
        V = lambda kt: (lambda: v_group(kt))
        O = lambda mt: (lambda: oproj_mt(mt))

        # head: k columns (all query blocks) + q columns for qb0 of pair 0
        for qb in range(N_QB):
            qk_group(wk_t, bk_t, kT_t, 0, qb)
        qk_group(wq_t, bq_t, qT_t, 0, 0)

        attention(0, 0, pre=(V(0), V(1)), fillers={
            **{kt: [V(kt + 2)] for kt in range(14)},
            14: [G(wq_t, bq_t, qT_t, 0, 1)],
            15: [G(wq_t, bq_t, qT_t, 0, 2)],
        })
        attention(0, 1, fillers={
            1: [G(wq_t, bq_t, qT_t, 0, 3)],
            4: [G(wk_t, bk_t, kT_t, 1, 0)],
            8: [G(wk_t, bk_t, kT_t, 1, 1)],
            12: [G(wk_t, bk_t, kT_t, 1, 2)],
        })
        attention(0, 2, fillers={
            2: [G(wk_t, bk_t, kT_t, 1, 3)],
            6: [G(wq_t, bq_t, qT_t, 1, 0)],
            10: [G(wq_t, bq_t, qT_t, 1, 1)],
            14: [G(wq_t, bq_t, qT_t, 1, 2)],
        })
        attention(0, 3, fillers={
            4: [G(wq_t, bq_t, qT_t, 1, 3)],
        })
        attention(1, 0)
        attention(1, 1, fillers={3: [O(0)], 7: [O(1)], 11: [O(2)], 15: [O(3)]})
        attention(1, 2, fillers={3: [O(4)], 7: [O(5)], 11: [O(6)], 15: [O(7)]})
        attention(1, 3, fillers={3: [O(8)], 7: [O(9)], 11: [O(10)], 15: [O(11)]})
        for mt in range(12, 16):
            oproj_mt(mt)


def _build():
    import concourse.bacc as bacc
    import concourse.tile as tile
    from concourse import mybir

    F32 = mybir.dt.float32
    BF16 = mybir.dt.bfloat16

    nc = bacc.Bacc(
        "TRN2", target_bir_lowering=False, debug=False, num_devices=8
    )
    t = {
        "xT": nc.dram_tensor("xT", (D, NTOK), BF16, kind="ExternalInput").ap(),
        "wqT": nc.dram_tensor("wqT", (D, DG), BF16, kind="ExternalInput").ap(),
        "wkT": nc.dram_tensor("wkT", (D, DG), BF16, kind="ExternalInput").ap(),
        "wvT": nc.dram_tensor("wvT", (D, DG), BF16, kind="ExternalInput").ap(),
        "woT": nc.dram_tensor("woT", (DG, D), BF16, kind="ExternalInput").ap(),
        "bq2": nc.dram_tensor(
            "bq2", (P, DG // P), F32, kind="ExternalInput"
        ).ap(),
        "bk2": nc.dram_tensor(
            "bk2", (P, DG // P), F32, kind="ExternalInput"
        ).ap(),
        "bvb": nc.dram_tensor("bvb", (P, DG), F32, kind="ExternalInput").ap(),
        "partial": nc.dram_tensor(
            "partial", (NTOK, D), F32, kind="ExternalOutput"
        ).ap(),
    }
    with tile.TileContext(nc) as tc:
        _emit(tc, t)
    nc.compile()
    return nc


_CACHE = {}


def _get_nc():
    if "nc" not in _CACHE:
        _CACHE["nc"] = _build()
    return _CACHE["nc"]


def make_in_maps(x, Wq, bq, Wk, bk, Wv, bv, Wo):
    """Per-core host-side sharding: core c -> batch c//4, head group c%4."""
    in_maps = []
    for c in range(8):
        b, g = divmod(c, 4)
        sl = slice(DG * g, DG * (g + 1))
        in_maps.append(
            {
                "xT": np.ascontiguousarray(x[b].T).astype(_BF16),
                "wqT": np.ascontiguousarray(Wq[sl].T).astype(_BF16),
                "wkT": np.ascontiguousarray(Wk[sl].T).astype(_BF16),
                "wvT": np.ascontiguousarray(Wv[sl].T).astype(_BF16),
                "woT": np.ascontiguousarray(Wo[:, sl].T).astype(_BF16),
                "bq2": np.ascontiguousarray(
                    bq[sl].reshape(DG // P, P).T
                ).astype(np.float32),
                "bk2": np.ascontiguousarray(
                    bk[sl].reshape(DG // P, P).T
                ).astype(np.float32),
                "bvb": np.ascontiguousarray(
                    np.broadcast_to(bv[sl][None, :], (P, DG))
                ).astype(np.float32),
            }
        )
    return in_maps


def kernel(x, Wq, bq, Wk, bk, Wv, bv, Wo, bo, _spmd_kwargs=None):
    from concourse.bass_utils import run_bass_kernel_spmd

    x, Wq, bq, Wk, bk, Wv, bv, Wo, bo = (
        np.asarray(a, np.float32) for a in (x, Wq, bq, Wk, bk, Wv, bv, Wo, bo)
    )
    nc = _get_nc()
    in_maps = make_in_maps(x, Wq, bq, Wk, bk, Wv, bv, Wo)
    res = run_bass_kernel_spmd(
        nc, in_maps, list(range(8)), **(_spmd_kwargs or {})
    )
    _CACHE["last_results"] = res
    out = np.empty((B, NTOK, D), np.float32)
    for b in range(B):
        acc = res.results[4 * b]["partial"].astype(np.float32).copy()
        for g in range(1, 4):
            acc += res.results[4 * b + g]["partial"]
        out[b] = acc + bo[None, :]
    return out
